# revision 7
# baseline (speedup 1.0000x reference)
"""Multi-head self-attention (B=2, S=2048, D=1024, H=16) on 8 Trainium2 cores.

Sharding: Megatron-style tensor parallelism on the head dimension.
Each core owns 2 heads (128 of the 1024 model dims):
  - Wq/Wk/Wv column-sharded: core c computes Q/K/V for dims [c*128,(c+1)*128)
  - attention for its 2 heads over both batches
  - Wo row-sharded: core c produces a partial output [4096, 1024] (bf16)
  - host sums the 8 partials and adds bo.

Engine-balance design (trace-driven):
  - Q/K projections and the QK^T score matmuls run in fp8e4 with DoubleRow
    perf mode (2 contraction rows per PE pass). Q/K quantization noise
    averages out through softmax+PV, so this is numerically cheap. V, PV,
    and both bf16 projections stay bf16 because their quantization error
    lands directly on the output.
  - scores-DR needs contraction d=64 split as [32 partitions x 2 pair]:
    Q/K are projected into a [128, T] fp8 staging tile, then 4 SB->SB
    X-bar DMAs rearrange each into [32, 2(dhalf), 2(head), T].
  - exp of the score matrix (16.8M elems/core, was 143us on ACT alone) is
    split between the scalar engine (true exp) and the vector engine
    using a Schraudolph bf16-bit trick: bits = rne(A*s + B) written as
    int16 and bitcast to bf16 for the PV matmul. The +0.9% mean bias of
    the Schraudolph tiles is replicated onto the ACT tiles via the
    activation bias (exp(scale*s + ln(1.009064))) so it cancels in the
    softmax normalization.
  - the ones-column folded into V turns the PV matmul into a fused
    context+denominator computation; softmax needs no max subtraction
    (scores*0.125 are ~N(0,1) for this problem family).
  - output partials are written bf16 (halves DVE copy + DMA traffic);
    the host accumulates in f64.
"""

import numpy as np
import ml_dtypes
from contextlib import ExitStack

import concourse.bass as bass
import concourse.tile as tile
from concourse import bacc, mybir
from concourse.bass_utils import run_bass_kernel_spmd
from concourse.masks import make_identity

B, S, D = 2, 2048, 1024
H, DH = 16, 64
T = B * S                  # 4096 tokens total
N_CORES = 8
OPC = D // N_CORES         # 128 out dims per core
HPC = H // N_CORES         # 2 heads per core
NI = D // 128              # 8 contraction chunks of 128
NI2 = NI // 2              # 4 DoubleRow contraction pairs
TCH = 512                  # projection token chunk
QCH = 512                  # attention q chunk
NQCH = S // QCH            # 4 per batch
NKT = S // 128             # 16 key tiles per batch
HW = DH + 2                # 66 cols per head in the v tile (data|ones|pad)
VW = HPC * HW              # 132

F32 = mybir.dt.float32
BF16 = mybir.dt.bfloat16
FP8 = mybir.dt.float8e4
I16 = mybir.dt.int16
EXP = mybir.ActivationFunctionType.Exp
DR = mybir.MatmulPerfMode.DoubleRow

MM_DT, MM_NP = BF16, ml_dtypes.bfloat16
FP8_NP = ml_dtypes.float8_e4m3

# Schraudolph exp-as-bf16-bits constants: bits = rne(A*s + B) approximates
# exp(0.125*s) with a zero-mean +-4.1% sawtooth (HW DVE f32->int16 is RNE,
# verified). Zero mean matters: a mean bias on DVE-exp'd tiles but not
# ACT-exp'd tiles would systematically mis-weight their keys in softmax.
A_SCH = 128.0 * 1.4426950408889634 * 0.125
B_SCH = 127.0 * 128.0 - 7.3

# which kt tiles (of 16 per q-chunk) run exp on the vector engine
DVE_KT = frozenset((1, 3, 5, 7, 9, 11, 13))


def _mha_kernel(tc, y, xT8, xT16, w8q, w8k, wv, woT, bq, bk, bv):
    with ExitStack() as ctx:
        _mha_kernel_inner(ctx, tc, y, xT8, xT16, w8q, w8k, wv, woT, bq, bk, bv)


def _mha_kernel_inner(ctx, tc, y, xT8, xT16, w8q, w8k, wv, woT, bq, bk, bv):
    nc = tc.nc
    pers = ctx.enter_context(tc.tile_pool(name="pers", bufs=1))

    qT8s = pers.tile([128, T], FP8, tag="qT8s")
    kT8s = pers.tile([128, T], FP8, tag="kT8s")
    qT8r = pers.tile([32, 2, HPC, T], FP8, tag="qT8r")
    kT8r = pers.tile([32, 2, HPC, T], FP8, tag="kT8r")
    vT = pers.tile([128, T], BF16, tag="vT")
    vtk = pers.tile([128, B * NKT, VW], MM_DT, tag="vtk")
    w8q_sb = pers.tile([128, NI2, 2, OPC], FP8, tag="w8q")
    w8k_sb = pers.tile([128, NI2, 2, OPC], FP8, tag="w8k")
    wv_sb = pers.tile([128, NI, OPC], MM_DT, tag="wv")
    woT_sb = pers.tile([128, D], MM_DT, tag="wo")
    bq_sb = pers.tile([128, 1], F32, tag="bq")
    bk_sb = pers.tile([128, 1], F32, tag="bk")
    bv_sb = pers.tile([128, 1], F32, tag="bv")
    ident = pers.tile([128, 128], BF16, tag="ident")

    # identity first: it shares the gpsimd queue with the weight DMAs
    # and the chunk-0 V transposes wait on it. One DMA per weight tensor.
    make_identity(nc, ident)
    nc.gpsimd.dma_start(w8q_sb[:, :, :, :], w8q[:, :, :, :])
    nc.gpsimd.dma_start(w8k_sb[:, :, :, :], w8k[:, :, :, :])
    nc.gpsimd.dma_start(wv_sb[:, :, :], wv[:, :, :])
    nc.gpsimd.dma_start(woT_sb, woT)
    nc.gpsimd.dma_start(bq_sb, bq)
    nc.gpsimd.dma_start(bk_sb, bk)
    nc.gpsimd.dma_start(bv_sb, bv)
    # constant ones/pad columns of vtk
    onepad = pers.tile([128, 2], F32, tag="onepad")
    nc.vector.memset(onepad[:, 0:1], 1.0)
    nc.vector.memset(onepad[:, 1:2], 0.0)
    onepad_b = bass.AP(
        tensor=onepad.tensor,
        offset=onepad.offset,
        ap=[onepad.ap[0], [0, B * NKT], onepad.ap[1]],
    )
    for h in range(HPC):
        nc.vector.tensor_copy(
            vtk[:, :, h * HW + DH : h * HW + DH + 2], onepad_b
        )

    # Phase A1: Q/K projections, fp8 DoubleRow (pairs of 128-contractions).
    with (
        tc.tile_pool(name="psA", bufs=2, space="PSUM") as psA,
        tc.tile_pool(name="xin8", bufs=8) as xin8,
    ):
        for m in range(T // 1024):  # 4 double-chunks of 1024 tokens
            xts = []
            for i2 in range(NI2):
                xt = xin8.tile([128, 2, 1024], FP8, tag="xt8")
                nc.sync.dma_start(xt, xT8[i2, :, :, m * 1024 : (m + 1) * 1024])
                xts.append(xt)
            for half in range(2):
                t0 = m * 1024 + half * 512
                ps_q = psA.tile([128, TCH], F32, tag="ps_q")
                ps_k = psA.tile([128, TCH], F32, tag="ps_k")
                for i2 in range(NI2):
                    xs = xts[i2][:, :, half * 512 : (half + 1) * 512]
                    st, sp = (i2 == 0), (i2 == NI2 - 1)
                    nc.tensor.matmul(
                        ps_q, w8q_sb[:, i2, :, :], xs, start=st, stop=sp, perf_mode=DR
                    )
                    nc.tensor.matmul(
                        ps_k, w8k_sb[:, i2, :, :], xs, start=st, stop=sp, perf_mode=DR
                    )
                sl = slice(t0, t0 + TCH)
                nc.vector.tensor_scalar_add(qT8s[:, sl], ps_q, bq_sb)
                nc.vector.tensor_scalar_add(kT8s[:, sl], ps_k, bk_sb)

        # rearrange Q/K into [32, dhalf, head, T] for DoubleRow scores
        for h in range(HPC):
            for j in range(2):
                r0 = h * DH + j * 32
                nc.gpsimd.dma_start(kT8r[:, j, h, :], kT8s[r0 : r0 + 32, :])
                nc.gpsimd.dma_start(qT8r[:, j, h, :], qT8s[r0 : r0 + 32, :])

        # Phase A2: V projection (bf16) + transpose to token-major k-tiles.
        with tc.tile_pool(name="xin16", bufs=4) as xin16:
            for m in range(T // 1024):
                xa = xin16.tile([128, 4, 1024], MM_DT, tag="xt16")
                nc.sync.dma_start(xa, xT16[0, :, :, m * 1024 : (m + 1) * 1024])
                xb = xin16.tile([128, 4, 1024], MM_DT, tag="xt16")
                nc.sync.dma_start(xb, xT16[1, :, :, m * 1024 : (m + 1) * 1024])
                for half in range(2):
                    t0 = m * 1024 + half * 512
                    ps_v = psA.tile([128, TCH], F32, tag="ps_v")
                    for i in range(NI):
                        blk, u = divmod(i, 4)
                        src = xa if blk == 0 else xb
                        nc.tensor.matmul(
                            ps_v,
                            wv_sb[:, i, :],
                            src[:, u, half * 512 : (half + 1) * 512],
                            start=(i == 0),
                            stop=(i == NI - 1),
                        )
                    sl = slice(t0, t0 + TCH)
                    nc.vector.tensor_scalar_add(vT[:, sl], ps_v, bv_sb)
                    for g in range(t0 // 128, (t0 + TCH) // 128):
                        ps_t = psA.tile([128, 128], BF16, tag="ps_t")
                        nc.tensor.transpose(
                            ps_t, vT[:, g * 128 : (g + 1) * 128], ident
                        )
                        for h in range(HPC):
                            nc.vector.tensor_copy(
                                vtk[:, g, h * HW : h * HW + DH],
                                ps_t[:, h * DH : (h + 1) * DH],
                            )

    # Phase B: attention + output projection.
    with (
        tc.tile_pool(name="psB", bufs=2, space="PSUM") as psB,
        tc.tile_pool(name="att", bufs=8) as att,
        tc.tile_pool(name="sm", bufs=3) as sm,
        tc.tile_pool(name="yo_p", bufs=4) as yo_p,
    ):
        for b in range(B):
            for qc in range(NQCH):
                q0 = b * S + qc * QCH
                ctx_sb = sm.tile([128, QCH], MM_DT, tag="ctx")
                at_tiles = []
                for kt in range(NKT):
                    g = b * NKT + kt
                    ps_s = psB.tile([128, 2, QCH], F32, tag="ps_s")
                    for h in range(HPC):
                        nc.tensor.matmul(
                            ps_s[:, h, :],
                            kT8r[:, :, h, g * 128 : (g + 1) * 128],
                            qT8r[:, :, h, q0 : q0 + QCH],
                            start=True,
                            stop=True,
                            perf_mode=DR,
                        )
                    at = att.tile([128, 2, QCH], I16, tag="at")
                    if kt in DVE_KT:
                        nc.vector.tensor_scalar(
                            at,
                            ps_s,
                            A_SCH,
                            B_SCH,
                            mybir.AluOpType.mult,
                            mybir.AluOpType.add,
                        )
                    else:
                        nc.scalar.activation(
                            at.bitcast(BF16), ps_s, EXP, scale=0.125
                        )
                    at_tiles.append(at)
                pvs = [
                    psB.tile([HW, QCH], F32, tag="ps_pv", name=f"pv{h}")
                    for h in range(HPC)
                ]
                for kt in range(NKT):
                    g = b * NKT + kt
                    for h in range(HPC):
                        nc.tensor.matmul(
                            pvs[h],
                            vtk[:, g, h * HW : (h + 1) * HW],
                            at_tiles[kt].bitcast(BF16)[:, h, :],
                            start=(kt == 0),
                            stop=(kt == NKT - 1),
                        )
                for h in range(HPC):
                    # normalize: ctx rows for this head = pv[0:64] * recip(pv[64])
                    rraw = sm.tile([1, QCH], F32, tag="rraw")
                    nc.vector.tensor_copy(rraw, pvs[h][DH : DH + 1, :])
                    rrow = sm.tile([1, QCH], F32, tag="rrow")
                    nc.vector.reciprocal_approx_fast(rrow, rraw)
                    nrm = sm.tile([DH, QCH], F32, tag="nrm")
                    nc.gpsimd.partition_broadcast(nrm, rrow)
                    nc.vector.tensor_mul(
                        ctx_sb[h * DH : (h + 1) * DH, :], pvs[h][0:DH, :], nrm
                    )
                for t4 in range(QCH // 128):
                    yo = yo_p.tile([128, D], BF16, tag="yo")
                    for nch in range(D // 512):
                        ps_o = psB.tile([128, 512], F32, tag="ps_o")
                        nc.tensor.matmul(
                            ps_o,
                            ctx_sb[:, t4 * 128 : (t4 + 1) * 128],
                            woT_sb[:, nch * 512 : (nch + 1) * 512],
                            start=True,
                            stop=True,
                        )
                        nc.vector.tensor_copy(yo[:, nch * 512 : (nch + 1) * 512], ps_o)
                    r0 = q0 + t4 * 128
                    nc.sync.dma_start(y[r0 : r0 + 128, :], yo)


_NC_CACHE = {}


def _build_nc(repeats=1):
    if repeats in _NC_CACHE:
        return _NC_CACHE[repeats]
    nc = bacc.Bacc("TRN2", target_bir_lowering=False, debug=False, num_devices=N_CORES)
    xT8 = nc.dram_tensor("xT8", [NI2, 128, 2, T], FP8, kind="ExternalInput").ap()
    xT16 = nc.dram_tensor("xT16", [2, 128, 4, T], MM_DT, kind="ExternalInput").ap()
    w8q = nc.dram_tensor("w8q", [128, NI2, 2, OPC], FP8, kind="ExternalInput").ap()
    w8k = nc.dram_tensor("w8k", [128, NI2, 2, OPC], FP8, kind="ExternalInput").ap()
    wv = nc.dram_tensor("wv", [128, NI, OPC], MM_DT, kind="ExternalInput").ap()
    woT = nc.dram_tensor("woT", [128, D], MM_DT, kind="ExternalInput").ap()
    bq = nc.dram_tensor("bq", [128, 1], F32, kind="ExternalInput").ap()
    bk = nc.dram_tensor("bk", [128, 1], F32, kind="ExternalInput").ap()
    bv = nc.dram_tensor("bv", [128, 1], F32, kind="ExternalInput").ap()
    y = nc.dram_tensor("y", [T, D], BF16, kind="ExternalOutput").ap()
    with tile.TileContext(nc) as tc:
        for _ in range(repeats):
            _mha_kernel(tc, y, xT8, xT16, w8q, w8k, wv, woT, bq, bk, bv)
    nc.compile()
    _NC_CACHE[repeats] = nc
    return nc


def _prep_in_maps(inputs):
    x = np.asarray(inputs["x"], np.float32)
    Wq = np.asarray(inputs["Wq"], np.float32)
    Wk = np.asarray(inputs["Wk"], np.float32)
    Wv = np.asarray(inputs["Wv"], np.float32)
    Wo = np.asarray(inputs["Wo"], np.float32)
    bq = np.asarray(inputs["bq"], np.float32)
    bk = np.asarray(inputs["bk"], np.float32)
    bv = np.asarray(inputs["bv"], np.float32)

    xTr = np.ascontiguousarray(x.reshape(T, D).T)  # [D, T]
    # xT8 [i2, p, j, t] = xTr[(2*i2+j)*128 + p, t]
    xT8_np = np.ascontiguousarray(
        xTr.reshape(NI2, 2, 128, T).transpose(0, 2, 1, 3)
    ).astype(FP8_NP)
    # xT16 [blk, p, u, t] = xTr[(blk*4+u)*128 + p, t]
    xT16_np = np.ascontiguousarray(
        xTr.reshape(2, 4, 128, T).transpose(0, 2, 1, 3)
    ).astype(MM_NP)

    def _w8_slice(W, c):
        # [128(p), NI2, 2, OPC]: [p, i2, j, o] = W[c*OPC+o, (2*i2+j)*128+p]
        A = np.ascontiguousarray(W[c * OPC : (c + 1) * OPC, :].T)  # [D, OPC]
        return np.ascontiguousarray(
            A.reshape(NI2, 2, 128, OPC).transpose(2, 0, 1, 3)
        ).astype(FP8_NP)

    def _wv_slice(W, c):
        A = np.ascontiguousarray(W[c * OPC : (c + 1) * OPC, :].T)  # [D, OPC]
        return np.ascontiguousarray(A.reshape(NI, 128, OPC).transpose(1, 0, 2)).astype(
            MM_NP
        )

    in_maps = []
    for c in range(N_CORES):
        sl = slice(c * OPC, (c + 1) * OPC)
        in_maps.append(
            {
                "xT8": xT8_np,
                "xT16": xT16_np,
                "w8q": _w8_slice(Wq, c),
                "w8k": _w8_slice(Wk, c),
                "wv": _wv_slice(Wv, c),
                "woT": np.ascontiguousarray(Wo[:, sl].T).astype(MM_NP),
                "bq": bq[sl].reshape(OPC, 1).copy(),
                "bk": bk[sl].reshape(OPC, 1).copy(),
                "bv": bv[sl].reshape(OPC, 1).copy(),
            }
        )
    return in_maps


def kernel(**inputs) -> np.ndarray:
    nc = _build_nc()
    in_maps = _prep_in_maps(inputs)
    res = run_bass_kernel_spmd(nc, in_maps, core_ids=list(range(N_CORES)))
    bo = np.asarray(inputs["bo"], np.float32)
    y = np.zeros((T, D), np.float64)
    for c in range(N_CORES):
        y += res.results[c]["y"].astype(np.float64)
    y = (y + bo).astype(np.float32)
    return y.reshape(B, S, D)


# revision 12
# speedup vs baseline: 1.4325x; 1.4325x over previous
"""Multi-head self-attention (B=2, S=2048, D=1024, H=16) on 8 Trainium2 cores.

Sharding: Megatron-style tensor parallelism on the head dimension.
Each core owns 2 heads (128 of the 1024 model dims):
  - Wq/Wk/Wv column-sharded: core c computes Q/K/V for dims [c*128,(c+1)*128)
  - attention for its 2 heads over both batches
  - Wo row-sharded: core c produces a partial output [4096, 1024] (bf16)
  - host sums the 8 partials and adds bo.

All matmuls are bf16 (fp32 accumulate). fp8 variants were measured and
rejected: any noise sigma on the softmax logits appears ~1:1 as relative
error on the output (the context is a weighted mean whose magnitude
shrinks by the same sqrt(N) as the noise), so fp8 Q/K (5% logit noise)
and Schraudolph DVE-exp (+-4%) both blow the 2e-2 error budget.

Performance structure (trace-driven):
  - phase B is software-pipelined: the PV matmuls of key-tile kt-PV_LAG
    are emitted between the score matmuls of kt, so the PE never sits
    idle waiting for exp. PE idle gaps are doubly bad: they also drop
    the PE's HAM clock from 2.4 GHz to 1.2 GHz.
  - exp of the score matrix runs entirely on the scalar engine
    ([128,2,512] tiles, one ACTIVATE per key tile); the vector engine
    handles biases, softmax normalization, and PSUM->SBUF output casts.
  - the ones-column folded into V makes the PV matmul also produce the
    softmax denominator; no max subtraction needed (scores*0.125 are
    ~N(0,1) for this problem family).
  - x is DMA'd in 8 big [128,4,1024] transfers (dma_start issue time is
    ~0.6us each on the queueing engine; 64 small loads cost ~38us of
    queue time in the original baseline).
  - output partials are written bf16; the host accumulates in f64.
"""

import numpy as np
import ml_dtypes
from contextlib import ExitStack

import concourse.bass as bass
import concourse.tile as tile
from concourse import bacc, mybir
from concourse.bass_utils import run_bass_kernel_spmd
from concourse.masks import make_identity

B, S, D = 2, 2048, 1024
H, DH = 16, 64
T = B * S                  # 4096 tokens total
N_CORES = 8
OPC = D // N_CORES         # 128 out dims per core
HPC = H // N_CORES         # 2 heads per core
NI = D // 128              # 8 contraction chunks of 128
TCH = 512                  # projection token chunk
QCH = 512                  # attention q chunk
NQCH = S // QCH            # 4 per batch
NKT = S // 128             # 16 key tiles per batch
HW = DH + 2                # 66 cols per head in the v tile (data|ones|pad)
VW = HPC * HW              # 132

F32 = mybir.dt.float32
BF16 = mybir.dt.bfloat16
EXP = mybir.ActivationFunctionType.Exp

MM_DT, MM_NP = BF16, ml_dtypes.bfloat16

PV_LAG = 2


def _mha_kernel(tc, y, xT16, wq, wk, wv, woT, bq, bk, bv):
    with ExitStack() as ctx:
        _mha_kernel_inner(ctx, tc, y, xT16, wq, wk, wv, woT, bq, bk, bv)


def _mha_kernel_inner(ctx, tc, y, xT16, wq, wk, wv, woT, bq, bk, bv):
    nc = tc.nc
    pers = ctx.enter_context(tc.tile_pool(name="pers", bufs=1))

    qT = pers.tile([128, T], MM_DT, tag="qT")
    kT = pers.tile([128, T], MM_DT, tag="kT")
    vT = pers.tile([128, T], MM_DT, tag="vT")
    vtk = pers.tile([128, B * NKT, VW], MM_DT, tag="vtk")
    wq_sb = pers.tile([128, NI, OPC], MM_DT, tag="wq")
    wk_sb = pers.tile([128, NI, OPC], MM_DT, tag="wk")
    wv_sb = pers.tile([128, NI, OPC], MM_DT, tag="wv")
    woT_sb = pers.tile([128, D], MM_DT, tag="wo")
    bq_sb = pers.tile([128, 1], F32, tag="bq")
    bk_sb = pers.tile([128, 1], F32, tag="bk")
    bv_sb = pers.tile([128, 1], F32, tag="bv")
    ident = pers.tile([128, 128], MM_DT, tag="ident")

    # identity first: it shares the gpsimd queue with the weight DMAs
    # and the chunk-0 V transposes wait on it. One DMA per weight tensor.
    make_identity(nc, ident)
    nc.gpsimd.dma_start(wq_sb[:, :, :], wq[:, :, :])
    nc.gpsimd.dma_start(wk_sb[:, :, :], wk[:, :, :])
    nc.gpsimd.dma_start(wv_sb[:, :, :], wv[:, :, :])
    nc.gpsimd.dma_start(woT_sb, woT)
    nc.gpsimd.dma_start(bq_sb, bq)
    nc.gpsimd.dma_start(bk_sb, bk)
    nc.gpsimd.dma_start(bv_sb, bv)
    # constant ones/pad columns of vtk
    onepad = pers.tile([128, 2], F32, tag="onepad")
    nc.vector.memset(onepad[:, 0:1], 1.0)
    nc.vector.memset(onepad[:, 1:2], 0.0)
    onepad_b = bass.AP(
        tensor=onepad.tensor,
        offset=onepad.offset,
        ap=[onepad.ap[0], [0, B * NKT], onepad.ap[1]],
    )
    for h in range(HPC):
        nc.vector.tensor_copy(
            vtk[:, :, h * HW + DH : h * HW + DH + 2], onepad_b
        )

    # Phase A: Q/K/V projections in o-major layout; V transposed to
    # token-major k-tiles right away.
    with (
        tc.tile_pool(name="psA", bufs=2, space="PSUM") as psA,
        tc.tile_pool(name="xin", bufs=4) as xin,
    ):
        for m in range(T // 1024):  # 4 double-chunks of 1024 tokens
            xa = xin.tile([128, 4, 1024], MM_DT, tag="xt")
            nc.sync.dma_start(xa, xT16[0, :, :, m * 1024 : (m + 1) * 1024])
            xb = xin.tile([128, 4, 1024], MM_DT, tag="xt")
            nc.sync.dma_start(xb, xT16[1, :, :, m * 1024 : (m + 1) * 1024])
            for half in range(2):
                t0 = m * 1024 + half * 512
                hs = slice(half * 512, (half + 1) * 512)
                ps_q = psA.tile([128, TCH], F32, tag="ps_q")
                ps_k = psA.tile([128, TCH], F32, tag="ps_k")
                ps_v = psA.tile([128, TCH], F32, tag="ps_v")
                for i in range(NI):
                    blk, u = divmod(i, 4)
                    xs = (xa if blk == 0 else xb)[:, u, hs]
                    st, sp = (i == 0), (i == NI - 1)
                    nc.tensor.matmul(ps_q, wq_sb[:, i, :], xs, start=st, stop=sp)
                    nc.tensor.matmul(ps_k, wk_sb[:, i, :], xs, start=st, stop=sp)
                    nc.tensor.matmul(ps_v, wv_sb[:, i, :], xs, start=st, stop=sp)
                sl = slice(t0, t0 + TCH)
                nc.vector.tensor_scalar_add(qT[:, sl], ps_q, bq_sb)
                nc.vector.tensor_scalar_add(kT[:, sl], ps_k, bk_sb)
                nc.vector.tensor_scalar_add(vT[:, sl], ps_v, bv_sb)
                for g in range(t0 // 128, (t0 + TCH) // 128):
                    ps_t = psA.tile([128, 128], MM_DT, tag="ps_t")
                    nc.tensor.transpose(ps_t, vT[:, g * 128 : (g + 1) * 128], ident)
                    for h in range(HPC):
                        nc.vector.tensor_copy(
                            vtk[:, g, h * HW : h * HW + DH],
                            ps_t[:, h * DH : (h + 1) * DH],
                        )

    # Phase B: attention + output projection, software-pipelined so the PE
    # stays dense (PV of kt-PV_LAG emitted between the score matmuls of kt).
    with (
        tc.tile_pool(name="psB", bufs=2, space="PSUM") as psB,
        tc.tile_pool(name="att", bufs=8) as att,
        tc.tile_pool(name="sm", bufs=3) as sm,
        tc.tile_pool(name="yo_p", bufs=4) as yo_p,
    ):
        for b in range(B):
            for qc in range(NQCH):
                q0 = b * S + qc * QCH
                ctx_sb = sm.tile([128, QCH], MM_DT, tag="ctx")
                at_tiles = []
                pvs = [
                    psB.tile([HW, QCH], F32, tag="ps_pv", name=f"pv{h}")
                    for h in range(HPC)
                ]

                def emit_pv(kt):
                    g = b * NKT + kt
                    for h in range(HPC):
                        nc.tensor.matmul(
                            pvs[h],
                            vtk[:, g, h * HW : (h + 1) * HW],
                            at_tiles[kt][:, h, :],
                            start=(kt == 0),
                            stop=(kt == NKT - 1),
                        )

                for kt in range(NKT):
                    g = b * NKT + kt
                    ps_s = psB.tile([128, 2, QCH], F32, tag="ps_s")
                    for h in range(HPC):
                        hs = slice(h * DH, (h + 1) * DH)
                        nc.tensor.matmul(
                            ps_s[:, h, :],
                            kT[hs, g * 128 : (g + 1) * 128],
                            qT[hs, q0 : q0 + QCH],
                            start=True,
                            stop=True,
                        )
                    at = att.tile([128, 2, QCH], MM_DT, tag="at")
                    nc.scalar.activation(at, ps_s, EXP, scale=0.125)
                    at_tiles.append(at)
                    if kt >= PV_LAG:
                        emit_pv(kt - PV_LAG)
                for kt in range(NKT - PV_LAG, NKT):
                    emit_pv(kt)

                for h in range(HPC):
                    # normalize: ctx rows for this head = pv[0:64] * recip(pv[64])
                    rraw = sm.tile([1, QCH], F32, tag="rraw")
                    nc.vector.tensor_copy(rraw, pvs[h][DH : DH + 1, :])
                    rrow = sm.tile([1, QCH], F32, tag="rrow")
                    nc.vector.reciprocal_approx_fast(rrow, rraw)
                    nrm = sm.tile([DH, QCH], F32, tag="nrm")
                    nc.gpsimd.partition_broadcast(nrm, rrow)
                    nc.vector.tensor_mul(
                        ctx_sb[h * DH : (h + 1) * DH, :], pvs[h][0:DH, :], nrm
                    )
                for t4 in range(QCH // 128):
                    yo = yo_p.tile([128, D], BF16, tag="yo")
                    for nch in range(D // 512):
                        ps_o = psB.tile([128, 512], F32, tag="ps_o")
                        nc.tensor.matmul(
                            ps_o,
                            ctx_sb[:, t4 * 128 : (t4 + 1) * 128],
                            woT_sb[:, nch * 512 : (nch + 1) * 512],
                            start=True,
                            stop=True,
                        )
                        nc.vector.tensor_copy(yo[:, nch * 512 : (nch + 1) * 512], ps_o)
                    r0 = q0 + t4 * 128
                    nc.sync.dma_start(y[r0 : r0 + 128, :], yo)


_NC_CACHE = {}


def _build_nc(repeats=1):
    if repeats in _NC_CACHE:
        return _NC_CACHE[repeats]
    nc = bacc.Bacc("TRN2", target_bir_lowering=False, debug=False, num_devices=N_CORES)
    xT16 = nc.dram_tensor("xT16", [2, 128, 4, T], MM_DT, kind="ExternalInput").ap()
    wq = nc.dram_tensor("wq", [128, NI, OPC], MM_DT, kind="ExternalInput").ap()
    wk = nc.dram_tensor("wk", [128, NI, OPC], MM_DT, kind="ExternalInput").ap()
    wv = nc.dram_tensor("wv", [128, NI, OPC], MM_DT, kind="ExternalInput").ap()
    woT = nc.dram_tensor("woT", [128, D], MM_DT, kind="ExternalInput").ap()
    bq = nc.dram_tensor("bq", [128, 1], F32, kind="ExternalInput").ap()
    bk = nc.dram_tensor("bk", [128, 1], F32, kind="ExternalInput").ap()
    bv = nc.dram_tensor("bv", [128, 1], F32, kind="ExternalInput").ap()
    y = nc.dram_tensor("y", [T, D], BF16, kind="ExternalOutput").ap()
    with tile.TileContext(nc) as tc:
        for _ in range(repeats):
            _mha_kernel(tc, y, xT16, wq, wk, wv, woT, bq, bk, bv)
    nc.compile()
    _NC_CACHE[repeats] = nc
    return nc


def _prep_in_maps(inputs):
    x = np.asarray(inputs["x"], np.float32)
    Wq = np.asarray(inputs["Wq"], np.float32)
    Wk = np.asarray(inputs["Wk"], np.float32)
    Wv = np.asarray(inputs["Wv"], np.float32)
    Wo = np.asarray(inputs["Wo"], np.float32)
    bq = np.asarray(inputs["bq"], np.float32)
    bk = np.asarray(inputs["bk"], np.float32)
    bv = np.asarray(inputs["bv"], np.float32)

    xTr = np.ascontiguousarray(x.reshape(T, D).T)  # [D, T]
    # xT16 [blk, p, u, t] = xTr[(blk*4+u)*128 + p, t]
    xT16_np = np.ascontiguousarray(
        xTr.reshape(2, 4, 128, T).transpose(0, 2, 1, 3)
    ).astype(MM_NP)

    def _w_slice(W, c):
        # [128(p), NI, OPC]: [p, i, o] = W[c*OPC+o, i*128+p]
        A = np.ascontiguousarray(W[c * OPC : (c + 1) * OPC, :].T)  # [D, OPC]
        return np.ascontiguousarray(A.reshape(NI, 128, OPC).transpose(1, 0, 2)).astype(
            MM_NP
        )

    in_maps = []
    for c in range(N_CORES):
        sl = slice(c * OPC, (c + 1) * OPC)
        in_maps.append(
            {
                "xT16": xT16_np,
                "wq": _w_slice(Wq, c),
                "wk": _w_slice(Wk, c),
                "wv": _w_slice(Wv, c),
                "woT": np.ascontiguousarray(Wo[:, sl].T).astype(MM_NP),
                "bq": bq[sl].reshape(OPC, 1).copy(),
                "bk": bk[sl].reshape(OPC, 1).copy(),
                "bv": bv[sl].reshape(OPC, 1).copy(),
            }
        )
    return in_maps


def kernel(**inputs) -> np.ndarray:
    nc = _build_nc()
    in_maps = _prep_in_maps(inputs)
    res = run_bass_kernel_spmd(nc, in_maps, core_ids=list(range(N_CORES)))
    bo = np.asarray(inputs["bo"], np.float32)
    y = np.zeros((T, D), np.float64)
    for c in range(N_CORES):
        y += res.results[c]["y"].astype(np.float64)
    y = (y + bo).astype(np.float32)
    return y.reshape(B, S, D)
